# revision 40
# baseline (speedup 1.0000x reference)
"""Trainium2 Bass kernel for 10-layer LSTM + additive attention pooling + FC.

Sharding: data-parallel over batch (8 cores x 32). Per core all 10 layers run
as a wavefront (layer l computes step t = tick - l). Gates are batch-major
[32b x 512] per layer; 4 layers share one PSUM bank stacked on partitions;
all matmul streams are fp16 (1 cycle/row on PE).

All four gates go through ONE Tanh activation per group: sigmoid(z) =
(tanh(z/2)+1)/2, with the /2 pre-folded into the i/f/o weight columns and the
(+1)/2 corrections folded into scalar_tensor_tensor cell ops. The cell state
is carried as C=2c and the hidden state as H=2h (the 2x is undone in the
attention/FC weights). Biases for each 4-layer group are applied with one K=4
indicator matmul per group per tick. Cell ops are split per group so ACT/DVE
overlap the next group's matmuls; h returns to H-major via one fp16 PE
transpose per group per tick (the last group's transpose is deferred past the
next tick's first matmul burst to keep PE fed).
"""
import sys
import numpy as np

B, S, IN, H, OUT, L = 256, 512, 27, 128, 7, 10
NCORES = 8
BC = B // NCORES  # 32
G4 = 4 * H        # 512

for _p in ("/opt/trn_rl_repo",):
    if _p not in sys.path:
        sys.path.insert(0, _p)

_CACHE = {}


def _build(S_run):
    from contextlib import ExitStack
    import concourse.bass as bass
    import concourse.tile as tile
    from concourse import bacc, mybir
    from concourse.masks import make_identity

    f32 = mybir.dt.float32
    fp16 = mybir.dt.float16
    f8 = mybir.dt.float8e4
    DR = mybir.MatmulPerfMode.DoubleRow
    NT = S_run + L - 1

    nc = bacc.Bacc("TRN2", target_bir_lowering=False, debug=False,
                   enable_asserts=False, num_devices=NCORES)

    d_x = nc.dram_tensor("x", [IN + 1, S_run * BC], fp16, kind="ExternalInput").ap()
    d_w0 = nc.dram_tensor("w0", [IN + 1, G4], fp16, kind="ExternalInput").ap()
    d_wx = nc.dram_tensor("wx", [128, 9 * G4], fp16, kind="ExternalInput").ap()
    d_wh = nc.dram_tensor("wh", [128, 10 * G4], fp16, kind="ExternalInput").ap()
    # bias pre-broadcast to all 128 partitions (copied into PSUM by DVE/ACT)
    d_bias = nc.dram_tensor("biasb", [128, 3 * G4], f32,
                            kind="ExternalInput").ap()
    # K=4 indicator bias matmul operands (group 0 rides on the PE to fill
    # the transpose-wait gap)
    d_bias4 = nc.dram_tensor("bias4", [4, 3 * G4], fp16,
                             kind="ExternalInput").ap()
    d_ind4 = nc.dram_tensor("ind4", [4, 128], fp16, kind="ExternalInput").ap()
    d_attn = nc.dram_tensor("attn_wT", [128, 128], fp16, kind="ExternalInput").ap()
    d_attnb = nc.dram_tensor("attn_b", [128, 1], f32, kind="ExternalInput").ap()
    d_vw = nc.dram_tensor("v_w", [128, 1], fp16, kind="ExternalInput").ap()
    d_fcw = nc.dram_tensor("fc_wT", [128, OUT], f32, kind="ExternalInput").ap()
    d_fcb = nc.dram_tensor("fc_b", [1, OUT], f32, kind="ExternalInput").ap()
    d_out = nc.dram_tensor("out", [OUT, BC], f32, kind="ExternalOutput").ap()
    import os as _os
    dbg = _os.environ.get("DEBUG_HS9") == "1"
    d_hs9 = (nc.dram_tensor("hs9", [128, S_run * BC], fp16,
                            kind="ExternalOutput").ap() if dbg else None)

    Tanh = mybir.ActivationFunctionType.Tanh
    Exp = mybir.ActivationFunctionType.Exp
    Mult = mybir.AluOpType.mult
    Add = mybir.AluOpType.add

    with tile.TileContext(nc) as tc:
        with ExitStack() as octx:
            keep = octx.enter_context(tc.tile_pool(name="keep", bufs=1))
            hs9 = keep.tile([128, S_run * BC], fp16)
            ident = keep.tile([128, 128], fp16)
            make_identity(nc, ident[:])

            # ================= recurrent phase =================
            with ExitStack() as ctx:
                stat = ctx.enter_context(tc.tile_pool(name="stat", bufs=1))
                xT = stat.tile([IN + 1, S_run * BC], fp16)
                nc.sync.dma_start(xT[:], d_x)
                w0 = stat.tile([IN + 1, G4], fp16)
                nc.sync.dma_start(w0[:], d_w0)
                Wx = stat.tile([128, 9 * G4], fp16)
                nc.sync.dma_start(Wx[:], d_wx)
                Wh = stat.tile([128, 10 * G4], fp16)
                nc.sync.dma_start(Wh[:], d_wh)
                biasb = stat.tile([128, 3 * G4], f32)
                nc.sync.dma_start(biasb[:], d_bias)
                bias4 = stat.tile([4, 3 * G4], fp16)
                nc.sync.dma_start(bias4[:], d_bias4)
                ind4 = stat.tile([4, 128], fp16)
                nc.sync.dma_start(ind4[:], d_ind4)

                psum = ctx.enter_context(tc.tile_pool(name="ps", bufs=2,
                                                      space="PSUM"))
                pst = ctx.enter_context(tc.tile_pool(name="pst", bufs=2,
                                                     space="PSUM"))
                actp = ctx.enter_context(tc.tile_pool(name="act", bufs=3))
                hbp = ctx.enter_context(tc.tile_pool(name="hb", bufs=3))
                htp = ctx.enter_context(tc.tile_pool(name="ht", bufs=3))
                tmpp = ctx.enter_context(tc.tile_pool(name="tmp", bufs=3))
                xbp = ctx.enter_context(tc.tile_pool(name="xb", bufs=2))
                thp = ctx.enter_context(tc.tile_pool(name="th", bufs=3))
                cpp = ctx.enter_context(tc.tile_pool(name="cp", bufs=1))

                c_t = cpp.tile([128, 384], f32)   # stores C = 2c
                nc.vector.memset(c_t[:], 0.0)
                hT_prev = None
                pending = []  # deferred (transpose, copy, hs9) emissions

                def cell(pc, ac, ti, tf, to, tg, cc, yy, xx, tth, hh, t0):
                    # pc holds z/2 for i,f,o columns and z for g (x2 overall
                    # scale folded into weights); one tanh covers all gates
                    nc.scalar.activation(ac, pc, Tanh, scale=0.5)
                    # xx = (tf+1)*C = 4*f*c   (DVE, the c critical path)
                    if not t0:
                        nc.vector.scalar_tensor_tensor(
                            xx, tf, 1.0, cc, Add, Mult)
                    # yy = (ti+1)*tg = 2*i*g
                    nc.vector.scalar_tensor_tensor(yy, ti, 1.0, tg, Add, Mult)
                    if t0:
                        nc.vector.tensor_copy(cc, yy)   # C = 2*i*g
                    else:
                        # C' = xx/2 + yy = 2*c_new
                        nc.vector.scalar_tensor_tensor(
                            cc, xx, 0.5, yy, Mult, Add)
                    nc.scalar.activation(tth, cc, Tanh, scale=0.5)
                    # hh = (to+1)*tanh(c) = 2*h
                    nc.vector.scalar_tensor_tensor(hh, to, 1.0, tth, Add, Mult)

                def cell_group(g, psg, act, tmp, xb, th, h_b):
                    pc = psg[:, 0:G4]
                    ac = act[:, G4 * g:G4 * (g + 1)]
                    cc = c_t[:, 128 * g:128 * (g + 1)]
                    yy = tmp[:, 128 * g:128 * (g + 1)]
                    xx = xb[:, 128 * g:128 * (g + 1)]
                    tth = th[:, 128 * g:128 * (g + 1)]
                    hh = h_b[:, 128 * g:128 * (g + 1)]
                    cell(pc, ac, ac[:, 0:128], ac[:, 128:256],
                         ac[:, 256:384], ac[:, 384:512],
                         cc, yy, xx, tth, hh, False)

                def cell_a(g, psg, act, tmp, xb):
                    # gate tanh + the c update (DVE)
                    pc = psg[:, 0:G4]
                    ac = act[:, G4 * g:G4 * (g + 1)]
                    cc = c_t[:, 128 * g:128 * (g + 1)]
                    yy = tmp[:, 128 * g:128 * (g + 1)]
                    xx = xb[:, 128 * g:128 * (g + 1)]
                    nc.scalar.activation(ac, pc, Tanh, scale=0.5)
                    nc.vector.scalar_tensor_tensor(
                        xx, ac[:, 128:256], 1.0, cc, Add, Mult)
                    nc.vector.scalar_tensor_tensor(
                        yy, ac[:, 0:128], 1.0, ac[:, 384:512], Add, Mult)
                    nc.vector.scalar_tensor_tensor(
                        cc, xx, 0.5, yy, Mult, Add)

                def cell_b(g, act, th, h_b):
                    # tanh(c) + h
                    ac = act[:, G4 * g:G4 * (g + 1)]
                    cc = c_t[:, 128 * g:128 * (g + 1)]
                    tth = th[:, 128 * g:128 * (g + 1)]
                    hh = h_b[:, 128 * g:128 * (g + 1)]
                    nc.scalar.activation(tth, cc, Tanh, scale=0.5)
                    nc.vector.scalar_tensor_tensor(
                        hh, ac[:, 256:384], 1.0, tth, Add, Mult)

                def emit_x(l, k, pg, last=False):
                    t = k - l
                    m = l % 4
                    pr = slice(32 * m, 32 * m + 32)
                    o = pg[pr, 0:G4]
                    tp = (0, 32 * m)
                    if l == 0:
                        lx, wx_r = xT[:, 32 * t:32 * t + 32], w0[:]
                    else:
                        lx = hT_prev[:, 32 * (l - 1):32 * (l - 1) + 32]
                        wx_r = Wx[:, (l - 1) * G4:l * G4]
                    nc.tensor.matmul(o, lx, wx_r, start=False, stop=last,
                                     skip_group_check=True, tile_position=tp)

                def emit_rec(l, k, pg):
                    if k - l <= 0:
                        return
                    m = l % 4
                    pr = slice(32 * m, 32 * m + 32)
                    nc.tensor.matmul(
                        pg[pr, 0:G4],
                        hT_prev[:, 32 * l:32 * l + 32],
                        Wh[:, l * G4:(l + 1) * G4],
                        start=False, stop=True, skip_group_check=True,
                        tile_position=(0, 32 * m))

                def emit_bias(g, pg):
                    # bias image copied into the group's PSUM bank before the
                    # matmuls accumulate onto it (start=False). Spread across
                    # DVE/ACT/PE... DVE and ACT write PSUM directly, keeping
                    # the PE's column groups free for the x/rec streams.
                    src = biasb[:, G4 * g:G4 * (g + 1)]
                    if g == 1:
                        nc.scalar.copy(pg[:, 0:G4], src)
                    else:
                        nc.vector.tensor_copy(pg[:, 0:G4], src)

                def emit_bias_pe(g, pg):
                    nc.tensor.matmul(
                        pg[:, 0:G4], ind4[:],
                        bias4[:, G4 * g:G4 * (g + 1)],
                        start=True, stop=False, skip_group_check=True)

                def emit_trcp(g, h_b, hT, k):
                    pt = pst.tile([128, 128], fp16)
                    nc.tensor.transpose(
                        pt[:], h_b[:, 128 * g:128 * (g + 1)], ident[:])
                    nc.vector.tensor_copy(
                        hT[:, 128 * g:128 * (g + 1)], pt[:])
                    if g == 2 and 0 <= k - 9 < S_run:
                        t9 = k - 9
                        nc.gpsimd.tensor_copy(
                            hs9[:, 32 * t9:32 * t9 + 32], hT[:, 288:320])

                def flush_pending():
                    while pending:
                        fn = pending.pop(0)
                        fn()

                GRP = ((0, 1, 2, 3), (4, 5, 6, 7), (8, 9))

                def alloc_pgs():
                    return [psum.tile([128, G4], f32, tag=f"g{g}",
                                      name=f"psg{g}")
                            for g in range(3)]

                pgs = alloc_pgs()
                for g in range(3):
                    emit_bias(g, pgs[g])
                pre_done = False
                for k in range(NT):
                    act_l = [l for l in range(L) if 0 <= k - l < S_run]
                    grouped = len(act_l) == L and k > 9
                    act_g = sorted(set(l // 4 for l in act_l))
                    act = actp.tile([128, 3 * G4], fp16)
                    tmp = tmpp.tile([128, 384], fp16)
                    xb = xbp.tile([128, 384], f32)
                    th = thp.tile([128, 384], fp16)
                    h_b = hbp.tile([128, 384], fp16)
                    hT = htp.tile([128, 384], fp16)

                    if grouped:
                        for l in GRP[0]:
                            if l == 0 and pre_done:
                                continue
                            emit_x(l, k, pgs[0])
                        for l in GRP[0]:
                            emit_rec(l, k, pgs[0])
                        # deferred g2 transpose of the previous tick runs on
                        # PE while cell(g2,k-1) drains
                        flush_pending()
                        for g in (1, 2):
                            for l in GRP[g]:
                                emit_x(l, k, pgs[g])
                            for l in GRP[g]:
                                emit_rec(l, k, pgs[g])
                        pgs_next = alloc_pgs()
                        # fill the PE's transpose-wait gap with next-tick
                        # work that needs no hT: group-0 bias (indicator
                        # matmul) and layer-0's x-part
                        emit_bias_pe(0, pgs_next[0])
                        if k + 1 <= S_run - 1:
                            emit_x(0, k + 1, pgs_next[0])
                            pre_done = True
                        else:
                            pre_done = False
                        # interleave: Tanh(g+1) slides between Tanh(g) and
                        # TanhC(g) on the ACT queue so neither blocks the
                        # other group's ready work
                        cell_a(0, pgs[0], act, tmp, xb)
                        cell_a(1, pgs[1], act, tmp, xb)
                        cell_b(0, act, th, h_b)
                        emit_trcp(0, h_b, hT, k)
                        cell_a(2, pgs[2], act, tmp, xb)
                        cell_b(1, act, th, h_b)
                        emit_trcp(1, h_b, hT, k)
                        emit_bias(1, pgs_next[1])
                        cell_b(2, act, th, h_b)
                        emit_bias(2, pgs_next[2])
                        pending.append(
                            lambda g=2, hb=h_b, ht=hT, kk=k:
                            emit_trcp(g, hb, ht, kk))
                    else:
                        for l in act_l:
                            emit_x(l, k, pgs[l // 4], last=(k - l == 0))
                        for l in act_l:
                            emit_rec(l, k, pgs[l // 4])
                        flush_pending()
                        pgs_next = alloc_pgs()
                        for l in act_l:
                            t = k - l
                            g, m = l // 4, l % 4
                            pr = slice(32 * m, 32 * m + 32)
                            pc = pgs[g][pr, 0:G4]
                            ac = act[pr, G4 * g:G4 * (g + 1)]
                            cc = c_t[pr, 128 * g:128 * (g + 1)]
                            yy = tmp[pr, 128 * g:128 * (g + 1)]
                            xx = xb[pr, 128 * g:128 * (g + 1)]
                            hc = th[pr, 128 * g:128 * (g + 1)]
                            bc = h_b[pr, 128 * g:128 * (g + 1)]
                            cell(pc, ac, ac[:, 0:128], ac[:, 128:256],
                                 ac[:, 256:384], ac[:, 384:512],
                                 cc, yy, xx, hc, bc, t == 0)
                        for g in act_g:
                            emit_trcp(g, h_b, hT, k)
                        for g in range(3):
                            emit_bias(g, pgs_next[g])
                    pgs = pgs_next
                    hT_prev = hT
                flush_pending()

            if d_hs9 is not None:
                nc.sync.dma_start(d_hs9, hs9[:])
            # ================= attention + FC =================
            # hs9 holds 2h; attn_wT and fc_wT are pre-divided by 2.
            with ExitStack() as ctx:
                st2 = ctx.enter_context(tc.tile_pool(name="st2", bufs=1))
                ps2 = ctx.enter_context(tc.tile_pool(name="ps2", bufs=2,
                                                     space="PSUM"))
                sc2 = ctx.enter_context(tc.tile_pool(name="sc2", bufs=2))
                aw = st2.tile([128, 128], fp16)
                nc.sync.dma_start(aw[:], d_attn)
                ab = st2.tile([128, 1], f32)
                nc.sync.dma_start(ab[:], d_attnb)
                vw = st2.tile([128, 1], fp16)
                nc.sync.dma_start(vw[:], d_vw)
                fcw = st2.tile([128, OUT], f32)
                nc.sync.dma_start(fcw[:], d_fcw)
                fcb = st2.tile([1, OUT], f32)
                nc.sync.dma_start(fcb[:], d_fcb)
                ones128 = st2.tile([1, 128], fp16)
                nc.vector.memset(ones128[:], 1.0)
                ones128f = st2.tile([1, 128], f32)
                nc.vector.memset(ones128f[:], 1.0)
                onesBC = st2.tile([1, BC], f32)
                nc.vector.memset(onesBC[:], 1.0)

                NCH = (S_run * BC) // 512
                wgt = st2.tile([1, S_run * BC], fp16)
                for ch in range(NCH):
                    cs = slice(512 * ch, 512 * (ch + 1))
                    pa = ps2.tile([128, 512], f32, tag="big")
                    nc.tensor.matmul(pa[:], aw[:], hs9[:, cs],
                                     start=True, stop=True)
                    sc = sc2.tile([128, 512], fp16)
                    nc.scalar.activation(sc[:], pa[:], Tanh, bias=ab[:])
                    pl = ps2.tile([1, 512], f32, tag="pl")
                    nc.tensor.matmul(pl[:], vw[:], sc[:],
                                     start=True, stop=True)
                    nc.scalar.activation(wgt[:, cs], pl[:], Exp)
                # unnormalized weighted sum + per-b normalization at the end
                sm = st2.tile([1, BC], f32)
                nc.vector.tensor_reduce(
                    sm[:], wgt[:].rearrange("p (t b) -> p b t", b=BC),
                    axis=mybir.AxisListType.X, op=mybir.AluOpType.add)
                rsm = st2.tile([1, BC], f32)
                nc.vector.reciprocal(rsm[:], sm[:])

                parts = st2.tile([128, NCH * BC], f32)
                for ch in range(NCH):
                    cs = slice(512 * ch, 512 * (ch + 1))
                    pw = ps2.tile([128, 512], f32, tag="big")
                    nc.tensor.matmul(pw[:], ones128[:], wgt[:, cs],
                                     start=True, stop=True)
                    wp = sc2.tile([128, 512], f32, tag="wp")
                    nc.vector.tensor_mul(wp[:], hs9[:, cs], pw[:])
                    nc.vector.tensor_reduce(
                        parts[:, BC * ch:BC * (ch + 1)],
                        wp[:].rearrange("p (t b) -> p b t", b=BC),
                        axis=mybir.AxisListType.X, op=mybir.AluOpType.add)
                ctxv = st2.tile([128, BC], f32)
                nc.vector.tensor_reduce(
                    ctxv[:], parts[:].rearrange("p (c b) -> p b c", b=BC),
                    axis=mybir.AxisListType.X, op=mybir.AluOpType.add)
                prn = ps2.tile([128, BC], f32, tag="pl")
                nc.tensor.matmul(prn[:], ones128f[:], rsm[:],
                                 start=True, stop=True)
                nc.vector.tensor_mul(ctxv[:], ctxv[:], prn[:])

                pf = ps2.tile([OUT, BC], f32, tag="pl")
                nc.tensor.matmul(pf[:], fcw[:], ctxv[:],
                                 start=True, stop=False)
                nc.tensor.matmul(pf[:], fcb[:], onesBC[:],
                                 start=False, stop=True)
                ov = sc2.tile([OUT, BC], f32, tag="ov")
                nc.vector.tensor_copy(ov[:], pf[:])
                nc.sync.dma_start(d_out, ov[:])

    nc.compile()
    return nc


def _prep_inputs(x, w_ih0, w_ih, w_hh, b_ih, b_hh, attn_w, attn_b, v_w, v_b,
                 fc_w, fc_b, S_run):
    f16 = np.float16
    perm = np.concatenate([np.arange(0, H), np.arange(H, 2 * H),
                           np.arange(3 * H, 4 * H), np.arange(2 * H, 3 * H)])
    # per-gate-column scale: i,f,o columns at z/2 relative to h-scale (H=2h),
    # g columns at z. For h-side weights: ifo /2, g x1; for the x side
    # (layer 0, unscaled x): ifo x1, g x2. Same for biases.
    colsc_h = np.concatenate([np.full(3 * H, 0.5, np.float32),
                              np.full(H, 1.0, np.float32)])
    colsc_x = np.concatenate([np.full(3 * H, 1.0, np.float32),
                              np.full(H, 2.0, np.float32)])
    w0 = np.concatenate([w_ih0.T, (b_ih[0] + b_hh[0])[None, :]],
                        0)[:, perm] * colsc_x
    wx = np.concatenate([w_ih[l - 1].T[:, perm] * colsc_h
                         for l in range(1, L)], 1)
    wh = np.concatenate([w_hh[l].T[:, perm] * colsc_h for l in range(L)], 1)
    # bias image broadcast to partitions: partition 32m+b of group g gets
    # layer 4g+m's bias row (zero for layer 0 / unused slots)
    biasb = np.zeros((128, 3 * G4), np.float32)
    for l in range(1, L):
        g, m = l // 4, l % 4
        biasb[32 * m:32 * (m + 1), g * G4:(g + 1) * G4] = (
            (b_ih[l] + b_hh[l])[perm] * colsc_x)[None, :]
    shared = {
        "w0": np.ascontiguousarray(w0).astype(f16),
        "wx": np.ascontiguousarray(wx).astype(f16),
        "wh": np.ascontiguousarray(wh).astype(f16),
        "biasb": np.ascontiguousarray(biasb, np.float32),
        "bias4": np.ascontiguousarray(
            np.stack([biasb[32 * m] for m in range(4)])).astype(f16),
        "ind4": np.ascontiguousarray(
            np.kron(np.eye(4, dtype=np.float32), np.ones((1, 32), np.float32))
        ).astype(f16),
        "attn_wT": np.ascontiguousarray(attn_w.T / 2.0).astype(f16),
        "attn_b": np.ascontiguousarray(attn_b[:, None], np.float32),
        "v_w": np.ascontiguousarray(v_w.T).astype(f16),
        "fc_wT": np.ascontiguousarray(fc_w.T / 2.0, np.float32),
        "fc_b": np.ascontiguousarray(fc_b[None, :], np.float32),
    }
    in_maps = []
    for c in range(NCORES):
        xs = x[c * BC:(c + 1) * BC, :S_run, :]
        xt = np.transpose(xs, (2, 1, 0)).reshape(IN, S_run * BC)
        xt = np.concatenate([xt, np.ones((1, S_run * BC), np.float32)], 0)
        m = dict(shared)
        m["x"] = np.ascontiguousarray(xt).astype(f16)
        in_maps.append(m)
    return in_maps


def run(inputs, S_run=S, trace=False):
    from concourse import bass_utils
    if S_run not in _CACHE:
        _CACHE[S_run] = _build(S_run)
    nc = _CACHE[S_run]
    in_maps = _prep_inputs(S_run=S_run, **inputs)
    res = bass_utils.run_bass_kernel_spmd(
        nc, in_maps, core_ids=list(range(NCORES)), trace=trace)
    out = np.concatenate([np.asarray(res.results[c]["out"], np.float32).T
                          for c in range(NCORES)], 0)
    return np.ascontiguousarray(out, np.float32), res


def kernel(**inputs):
    inputs = {k: np.asarray(v, np.float32) for k, v in inputs.items()}
    out, _ = run(inputs, S_run=S)
    return out
